# revision 9
# baseline (speedup 1.0000x reference)
"""Trainium2 Bass kernel for nn_DecomNet_RTV.

Self-contained: takes FULL inputs (as produced by setup_inputs()), shards the
batch of 8 images across 8 NeuronCores (1 image/core, pure data parallel),
runs a Bass/Tile kernel per core, gathers full outputs.

Algorithm notes (verified against the reference in fp64 on host):
  * The periodic FFT solve ifft2(fft2(rhs)/(1+mu*eigDtD)).real is computed
    with the separable real Discrete Hartley Transform: since the transfer
    function T(u,v)=1/(1+mu(a_u+b_v)) is even in u and v, the solve is
    I = Hm @ (T' o (Hm @ rhs @ Hm)) @ Hm with Hm = cos+sin (symmetric) and
    T' = T/N^2.  Each side-multiply maps to PE matmuls with the DATA as the
    stationary operand (out = lhsT.T @ mov), which chains the four stages
    with no transposes.
  * Gradients: DyI comes from an extra stage-4 pass with moving operand
    My = Hm @ (roll-matrix); DxI = roll(I,-1,0)-I is obtained by re-running
    stage 4 with the stationary operand W1 shifted by one along its free dim
    (a 513-wide padded copy supplies the wraparound column), minus I.
  * Shrinkage weights: the update gate mu*zc/(2a+mu*zc) with
    zc=max(|D|^(2-p),EPS) equals sigmoid(ln(mu) + max((2-p)ln|D|, lnEPS)
    - ln(2a)), computed with ACT Ln/Sigmoid — no pow, no reciprocal.
  * 3x3 'SAME' convs for alpha/px/py: banded-matrix matmuls over
    column-shifted copies of O; BN+sigmoid folded into one ACT pass.
  * mu (HyPaNet on a constant input) is computed on host; per-iteration
    T' maps and -mu_i-scaled shift constants are streamed from HBM.
"""

import os
import sys
import numpy as np

for _p in ("/opt/trn_rl_repo",):
    if _p not in sys.path and os.path.isdir(_p):
        sys.path.insert(0, _p)

N = 512
K1 = 10
EPS = 1e-3
NCORES = 8
LN_EPS = float(np.log(EPS))
LN_TINY = 1e-30

_COMPILED = {}


# ----------------------------------------------------------------------------
# host-side math
# ----------------------------------------------------------------------------

def _softplus(x):
    return np.log1p(np.exp(-np.abs(x))) + np.maximum(x, 0)


def _host_mu(inputs):
    w1 = np.asarray(inputs['hp_w1'], np.float64)
    b1 = np.asarray(inputs['hp_b1'], np.float64)
    w2 = np.asarray(inputs['hp_w2'], np.float64)
    b2 = np.asarray(inputs['hp_b2'], np.float64)
    w3 = np.asarray(inputs['hp_w3'], np.float64)
    b3 = np.asarray(inputs['hp_b3'], np.float64)
    h = np.maximum(w1[:, 0] + b1, 0)
    h2 = np.maximum(w2 @ h + b2, 0)
    return _softplus(w3 @ h2 + b3) + 1.0  # (10,)


def _host_consts(mu):
    n = np.arange(N)
    th = 2 * np.pi * np.outer(n, n) / N
    Hm = (np.cos(th) + np.sin(th)).astype(np.float64)
    a = (2 - 2 * np.cos(2 * np.pi * n / N)).astype(np.float64)
    eye = np.eye(N)
    Sp = np.roll(eye, 1, axis=0)     # Sp@p = roll(p, 1, axis=0)
    Sm = np.roll(eye, -1, axis=0)    # Sm@I = roll(I, -1, axis=0)
    Gx = (Sp.T - eye) @ Hm
    Kc = Sm.T - eye                  # I@Kc = roll(I,-1,axis=1) - I
    My = Hm @ Kc
    Tp = np.stack([1.0 / ((1.0 + m * (a[:, None] + a[None, :])) * (N * N))
                   for m in mu])     # (10,N,N)
    Gxs = np.stack([-mu[i] * Gx for i in range(1, K1)])  # (9,N,N)
    return Hm.astype(np.float32), My.astype(np.float32), \
        Tp.astype(np.float32), Gxs.astype(np.float32)


def _conv_banded(inputs):
    """CW[(j,dc)] = Vk_dc^T for conv j; (9,N,N) f32, plus per-conv folded
    BN scale/bias."""
    CW = np.zeros((9, N, N), np.float32)
    scales = []
    biases = []
    for j, pref in enumerate(("p1", "p2", "p3")):
        k = np.asarray(inputs[f"{pref}_w"], np.float64)[0, 0]  # (3,3)
        for dci, dc in enumerate((-1, 0, 1)):
            Vk = np.zeros((N, N), np.float64)
            for dr in (-1, 0, 1):
                idx = np.arange(max(0, -dr), min(N, N - dr))
                Vk[idx, idx + dr] = k[dr + 1, dc + 1]
            CW[3 * j + dci] = Vk.T.astype(np.float32)
        g = float(np.asarray(inputs[f"{pref}_g"])[0])
        rv = float(np.asarray(inputs[f"{pref}_rv"])[0])
        rm = float(np.asarray(inputs[f"{pref}_rm"])[0])
        beta = float(np.asarray(inputs[f"{pref}_beta"])[0])
        bcv = float(np.asarray(inputs[f"{pref}_b"])[0])
        s = g / np.sqrt(rv + 1e-5)
        scales.append(s)
        biases.append((bcv - rm) * s + beta)
    return CW, scales, biases


def _host_scal(mu, scales, biases):
    """(128, 40) broadcast scalar table."""
    cols = np.zeros(40, np.float32)
    cols[0:10] = np.log(mu)
    cols[10:19] = mu[1:10]            # +mu_{i+1} for tail of iter i
    cols[19:28] = -mu[1:10]           # -mu_{i+1}
    cols[28:31] = scales
    cols[31:34] = biases
    cols[34] = 2e-6      # ln(2*alpha) bias
    cols[35] = LN_TINY   # ln(|D| + tiny) bias
    return np.tile(cols[None, :], (128, 1)).astype(np.float32)


# ----------------------------------------------------------------------------
# device program
# ----------------------------------------------------------------------------

def _build_program():
    import concourse.bacc as bacc
    import concourse.tile as tile
    from concourse import mybir

    f32 = mybir.dt.float32
    f32r = mybir.dt.float32r
    AO = mybir.AluOpType
    AF = mybir.ActivationFunctionType

    nc = bacc.Bacc("TRN2", target_bir_lowering=False, debug=False,
                   enable_asserts=True, num_devices=NCORES)

    # DRAM I/O -------------------------------------------------------------
    dO = nc.dram_tensor("O", [N, N], f32r, kind="ExternalInput").ap()
    dHm = nc.dram_tensor("Hm", [N, N], f32r, kind="ExternalInput").ap()
    dMy = nc.dram_tensor("My", [N, N], f32r, kind="ExternalInput").ap()
    dTp = nc.dram_tensor("Tp", [K1, N, N], f32, kind="ExternalInput").ap()
    dGxs = nc.dram_tensor("Gxs", [K1 - 1, N, N], f32r, kind="ExternalInput").ap()
    dCW = nc.dram_tensor("CW", [9, N, N], f32r, kind="ExternalInput").ap()
    dscal = nc.dram_tensor("scal", [128, 40], f32, kind="ExternalInput").ap()

    dI = nc.dram_tensor("Iout", [K1, N, N], f32, kind="ExternalOutput").ap()
    dR = nc.dram_tensor("Rout", [N, N], f32, kind="ExternalOutput").ap()
    dA = nc.dram_tensor("Aout", [N, N], f32, kind="ExternalOutput").ap()
    dPX = nc.dram_tensor("PXout", [N, N], f32, kind="ExternalOutput").ap()
    dPY = nc.dram_tensor("PYout", [N, N], f32, kind="ExternalOutput").ap()

    def drv(ap):  # DRAM (N,N) -> (128, 4, 512) view matching SBUF layout
        return ap.rearrange("(t p) c -> p t c", p=128)

    with tile.TileContext(nc) as tc:
        with tc.tile_pool(name="persist", bufs=1) as PP:
            O_sb = PP.tile([128, 2048], f32r, name="O_sb")
            Hm_sb = PP.tile([128, 2048], f32r, name="Hm_sb")
            My_sb = PP.tile([128, 2048], f32r, name="My_sb")
            scal = PP.tile([128, 40], f32, name="scal")
            lnal = PP.tile([128, 2048], f32, name="lnal")
            p2x = PP.tile([128, 2048], f32, name="p2x")
            p2y = PP.tile([128, 2048], f32, name="p2y")
            d1 = PP.tile([128, 2048], f32, name="d1")
            d2 = PP.tile([128, 2048], f32, name="d2")
            w1 = PP.tile([128, 2048], f32, name="w1")
            w2 = PP.tile([128, 2048], f32, name="w2")
            tp_sb = [PP.tile([128, 2048], f32, name=f"tp{j}") for j in range(2)]
            gx_sb = [PP.tile([128, 2048], f32r, name=f"gx{j}") for j in range(2)]

            def v3(t, width=512):   # (128, blocks, width) view
                return t.rearrange("p (t c) -> p t c", c=width)

            nc.sync.dma_start(out=v3(O_sb), in_=drv(dO))
            nc.sync.dma_start(out=v3(Hm_sb), in_=drv(dHm))
            nc.sync.dma_start(out=v3(My_sb), in_=drv(dMy))
            nc.sync.dma_start(out=scal[:], in_=dscal)
            nc.vector.memset(w1[:], 0.0)
            nc.vector.memset(w2[:], 0.0)

            def sc(col):  # (128,1) scalar AP from the scal table
                return scal[:, col:col + 1]

            # ---------------- prologue: convs -> alpha/px/py ----------------
            sig_sb = []
            with tc.tile_pool(name="prolog", bufs=1) as QP, \
                 tc.tile_pool(name="ppsum", bufs=4, space="PSUM") as QPS:
                # column-shifted copies of O (zero 'SAME' padding)
                odc = {}
                for dci, dc in ((0, -1), (2, 1)):
                    t = QP.tile([128, 2048], f32r, name=f"odc{dci}", tag=f"odc{dci}")
                    nc.vector.memset(t.bitcast(f32)[:], 0.0)
                    if dc == 1:
                        nc.sync.dma_start(out=v3(t)[:, :, 0:511],
                                          in_=drv(dO)[:, :, 1:512])
                    else:
                        nc.sync.dma_start(out=v3(t)[:, :, 1:512],
                                          in_=drv(dO)[:, :, 0:511])
                    odc[dci] = t
                odc[1] = O_sb

                for j in range(3):
                    sig = QP.tile([128, 2048], f32, name=f"sig{j}",
                                  tag=f"sig{j}")
                    sig_sb.append(sig)
                    cw = []
                    for dci in range(3):
                        c = QP.tile([128, 2048], f32r, name=f"cw{dci}",
                                    tag=f"cw{dci}")
                        nc.sync.dma_start(out=v3(c), in_=drv(dCW[3 * j + dci]))
                        cw.append(c)
                    for mc in range(4):
                        ps = QPS.tile([128, 512], f32, name="cps", tag="cps")
                        first = True
                        for kt in range(4):
                            for dci in range(3):
                                nc.tensor.matmul(
                                    ps[:],
                                    lhsT=cw[dci][:, 512 * kt + 128 * mc:
                                                 512 * kt + 128 * mc + 128],
                                    rhs=odc[dci][:, 512 * kt:512 * kt + 512],
                                    start=first,
                                    stop=(kt == 3 and dci == 2))
                                first = False
                        nc.scalar.activation(sig[:, 512 * mc:512 * mc + 512],
                                             ps[:], AF.Sigmoid,
                                             bias=sc(31 + j), scale=sc(28 + j))
                # outputs + derived fields
                aout = QP.tile([128, 2048], f32, name="aout", tag="aout")
                nc.vector.tensor_scalar(aout[:], sig_sb[0][:], 0.001, 1e-6,
                                        AO.mult, AO.add)
                nc.sync.dma_start(out=drv(dA), in_=v3(aout))
                pxo = QP.tile([128, 2048], f32, name="pxo", tag="pxo")
                nc.vector.tensor_scalar(pxo[:], sig_sb[1][:], 0.001, None, AO.add)
                nc.sync.dma_start(out=drv(dPX), in_=v3(pxo))
                pyo = QP.tile([128, 2048], f32, name="pyo", tag="pyo")
                nc.vector.tensor_scalar(pyo[:], sig_sb[2][:], 0.001, None, AO.add)
                nc.sync.dma_start(out=drv(dPY), in_=v3(pyo))
                nc.vector.tensor_scalar(p2x[:], sig_sb[1][:], -1.0, 1.999,
                                        AO.mult, AO.add)
                nc.vector.tensor_scalar(p2y[:], sig_sb[2][:], -1.0, 1.999,
                                        AO.mult, AO.add)
                nc.scalar.activation(lnal[:], sig_sb[0][:], AF.Ln,
                                     bias=sc(34), scale=0.002)

            # stream prefetch
            nc.sync.dma_start(out=v3(tp_sb[0]), in_=drv(dTp[0]))
            nc.sync.dma_start(out=v3(tp_sb[1]), in_=drv(dTp[1]))
            nc.sync.dma_start(out=v3(gx_sb[1]), in_=drv(dGxs[0]))   # iter 1
            nc.sync.dma_start(out=v3(gx_sb[0]), in_=drv(dGxs[1]))   # iter 2

            # ---------------- main loop ----------------
            with tc.tile_pool(name="work", bufs=1) as WP, \
                 tc.tile_pool(name="mpsum", bufs=8, space="PSUM") as PS:

                def wt(tag, shape=(128, 2048), dtype=f32):
                    return WP.tile(list(shape), dtype, name=tag, tag=tag)

                def stage(lhsT, mov, consume, lhs_off=0, lhs_stride=512,
                          extra=None):
                    """16-MM stage; consume(mc, psum_tile) per chunk.
                    extra=(lhsT2, mov2) accumulates a second 16-MM pass."""
                    for mc in range(4):
                        ps = PS.tile([128, 512], f32, name="ps", tag="ps")
                        for kt in range(4):
                            base = lhs_stride * kt + 128 * mc + lhs_off
                            nc.tensor.matmul(
                                ps[:],
                                lhsT=lhsT[:, base:base + 128],
                                rhs=mov[:, 512 * kt:512 * kt + 512],
                                start=(kt == 0),
                                stop=(extra is None and kt == 3))
                        if extra is not None:
                            l2, m2 = extra
                            for kt in range(4):
                                base = 512 * kt + 128 * mc
                                nc.tensor.matmul(
                                    ps[:],
                                    lhsT=l2[:, base:base + 128],
                                    rhs=m2[:, 512 * kt:512 * kt + 512],
                                    start=False, stop=(kt == 3))
                        consume(mc, ps)

                rhsp_t = O_sb
                p_t = None
                I_sb = None
                for i in range(K1):
                    lnmu = sc(i)
                    # ---- S1: Y^T ----
                    YT = wt("slotA", dtype=f32r)

                    def s1_consume(mc, ps):
                        nc.scalar.copy(YT[:, 512 * mc:512 * mc + 512], ps[:])

                    extra = None if i == 0 else (p_t, gx_sb[i % 2])
                    stage(rhsp_t, Hm_sb, s1_consume, extra=extra)
                    if 1 <= i <= 7:
                        nc.sync.dma_start(out=v3(gx_sb[i % 2]),
                                          in_=drv(dGxs[i + 1]))

                    # ---- S2: U -> V ----
                    V = wt("slotB", dtype=f32r)
                    tpb = tp_sb[i % 2]

                    def s2_consume(mc, ps):
                        nc.vector.tensor_tensor(
                            V[:, 512 * mc:512 * mc + 512], ps[:],
                            tpb[:, 512 * mc:512 * mc + 512], AO.mult)

                    stage(YT, Hm_sb, s2_consume)
                    if i + 2 < K1:
                        nc.sync.dma_start(out=v3(tp_sb[i % 2]),
                                          in_=drv(dTp[i + 2]))

                    # ---- S3: W1 (padded to 513-wide blocks) ----
                    W1p = wt("w1pad", (128, 4 * 513), dtype=f32r)

                    def s3_consume(mc, ps):
                        nc.scalar.copy(W1p[:, 513 * mc:513 * mc + 512], ps[:])

                    stage(V, Hm_sb, s3_consume)
                    w1p3 = W1p.rearrange("p (t c) -> p t c", c=513)
                    nc.scalar.copy(w1p3[:, :, 512:513], w1p3[:, :, 0:1])

                    # ---- S4: I ----
                    I_sb = wt("slotA")

                    def s4i_consume(mc, ps):
                        nc.scalar.copy(I_sb[:, 512 * mc:512 * mc + 512], ps[:])

                    stage(W1p, Hm_sb, s4i_consume, lhs_stride=513)
                    nc.sync.dma_start(out=drv(dI[i]), in_=v3(I_sb))

                    # ---- S4: DyI (psum kept) + y-chain ----
                    psd = []

                    def s4y_consume(mc, ps):
                        psd.append(ps)

                    stage(W1p, My_sb, s4y_consume, lhs_stride=513)
                    numy = wt("numy")
                    t_y = wt("ty")
                    for mc in range(4):
                        nc.vector.tensor_tensor(
                            numy[:, 512 * mc:512 * mc + 512], psd[mc][:],
                            w2[:, 512 * mc:512 * mc + 512], AO.add)
                        nc.scalar.activation(t_y[:, 512 * mc:512 * mc + 512],
                                             psd[mc][:], AF.Abs)
                    nc.scalar.activation(t_y[:], t_y[:], AF.Ln, bias=sc(35))
                    nc.vector.tensor_tensor(t_y[:], t_y[:], p2y[:], AO.mult)
                    nc.vector.scalar_tensor_tensor(
                        t_y[:], t_y[:], LN_EPS, lnal[:], AO.max, AO.subtract)

                    # ---- S4: Ish -> DxI + x-chain ----
                    psx = []

                    def s4x_consume(mc, ps):
                        psx.append(ps)

                    stage(W1p, Hm_sb, s4x_consume, lhs_off=1, lhs_stride=513)
                    dx = wt("slotB")
                    for mc in range(4):
                        nc.vector.tensor_tensor(
                            dx[:, 512 * mc:512 * mc + 512], psx[mc][:],
                            I_sb[:, 512 * mc:512 * mc + 512], AO.subtract)
                    numx = wt("numx")
                    nc.vector.tensor_tensor(numx[:], dx[:], w1[:], AO.add)
                    t_x = wt("tx")
                    nc.scalar.activation(t_x[:], dx[:], AF.Abs)
                    nc.scalar.activation(t_x[:], t_x[:], AF.Ln, bias=sc(35))
                    nc.vector.tensor_tensor(t_x[:], t_x[:], p2x[:], AO.mult)
                    nc.vector.scalar_tensor_tensor(
                        t_x[:], t_x[:], LN_EPS, lnal[:], AO.max, AO.subtract)

                    # ---- gates (grouped so ACT does Ln,Ln,Sig,Sig) ----
                    g_y = wt("gy")
                    nc.scalar.activation(g_y[:], t_y[:], AF.Sigmoid, bias=lnmu)
                    g_x = wt("gxt")
                    nc.scalar.activation(g_x[:], t_x[:], AF.Sigmoid, bias=lnmu)

                    # ---- state updates ----
                    nc.vector.tensor_tensor(d2[:], numy[:], g_y[:], AO.mult)
                    nc.vector.tensor_tensor(w2[:], numy[:], d2[:], AO.subtract)
                    nc.vector.tensor_tensor(d1[:], numx[:], g_x[:], AO.mult)
                    nc.vector.tensor_tensor(w1[:], numx[:], d1[:], AO.subtract)

                    # ---- tail: rhsp / p for next iteration ----
                    if i + 1 < K1:
                        p_t = wt("ty", dtype=f32r)
                        nc.vector.tensor_tensor(p_t[:], d1[:], w1[:],
                                                AO.subtract)
                        q_t = wt("slotB")
                        nc.vector.tensor_tensor(q_t[:], d2[:], w2[:],
                                                AO.subtract)
                        ta = wt("numy", dtype=f32r)
                        q3 = v3(q_t)
                        t3 = v3(ta)
                        O3 = v3(O_sb.bitcast(f32))
                        taf = ta.bitcast(f32)
                        mneg = sc(19 + i)
                        mpos = sc(10 + i)
                        nc.vector.scalar_tensor_tensor(
                            t3[:, :, 1:512], q3[:, :, 0:511], mneg,
                            O3[:, :, 1:512], AO.mult, AO.add)
                        nc.vector.scalar_tensor_tensor(
                            t3[:, :, 0:1], q3[:, :, 511:512], mneg,
                            O3[:, :, 0:1], AO.mult, AO.add)
                        nc.vector.scalar_tensor_tensor(
                            ta[:], q_t[:], mpos, taf[:], AO.mult, AO.add)
                        rhsp_t = ta

                # ---- epilogue: R = O / (I + EPS) ----
                t0 = wt("numx")
                nc.vector.tensor_scalar(t0[:], I_sb[:], EPS, None, AO.add)
                rr = wt("numy")
                scr = wt("tx")
                nc.vector.reciprocal_approx_accurate(rr[:], t0[:], scr[:])
                Rt = wt("slotB")
                nc.vector.tensor_tensor(Rt[:], rr[:], O_sb.bitcast(f32)[:], AO.mult)
                nc.sync.dma_start(out=drv(dR), in_=v3(Rt))

    nc.compile()
    return nc


def _get_program():
    if "nc" not in _COMPILED:
        _COMPILED["nc"] = _build_program()
    return _COMPILED["nc"]


# ----------------------------------------------------------------------------
# entry point
# ----------------------------------------------------------------------------

def kernel(**inputs):
    from concourse.bass_utils import run_bass_kernel_spmd

    O = np.asarray(inputs['O'], np.float32)
    B = O.shape[0]
    assert O.shape == (8, 1, N, N), O.shape

    mu = _host_mu(inputs)
    Hm, My, Tp, Gxs = _host_consts(mu)
    CW, scales, biases = _conv_banded(inputs)
    scal = _host_scal(mu.astype(np.float32), scales, biases)

    shared = {"Hm": Hm, "My": My, "Tp": Tp, "Gxs": Gxs, "CW": CW,
              "scal": scal}
    in_maps = [dict(shared, O=np.ascontiguousarray(O[b, 0])) for b in range(B)]

    nc = _get_program()
    res = run_bass_kernel_spmd(nc, in_maps, list(range(NCORES)))

    I_stack = np.zeros((K1, B, 1, N, N), np.float32)
    R = np.zeros((B, 1, N, N), np.float32)
    alpha = np.zeros((B, 1, N, N), np.float32)
    px = np.zeros((B, 1, N, N), np.float32)
    py = np.zeros((B, 1, N, N), np.float32)
    for b in range(B):
        r = res.results[b]
        I_stack[:, b, 0] = r["Iout"]
        R[b, 0] = r["Rout"]
        alpha[b, 0] = r["Aout"]
        px[b, 0] = r["PXout"]
        py[b, 0] = r["PYout"]
    mu_out = mu.astype(np.float32).reshape(1, K1, 1, 1)
    return I_stack, R, alpha, px, py, mu_out
